# revision 20
# baseline (speedup 1.0000x reference)
"""Weighted-MSE loss (Euler-angle + attribute weights) on 8 trn2 NeuronCores.

loss = mean(weight * (inp - label)^2),
  weight[i] = (sum_j 1-cos(ea[i,j])) * (sum_c attribute[i,c] * inv_freq[c])

Pure data-parallel over the batch dim; each of the 8 cores gets 4096 rows
(32 segments of 512 columns per SBUF partition: partition p holds rows
p*32..p*32+31). label is negated on host so every subtract is an ADD.

Structure (settled by profiling several designs on hardware):
- All DMAs ride the sync ring (HWDGE issued by the otherwise-idle Sync
  engine, ~390 GB/s aggregate). DMAs issued via the scalar ring
  serialize with ACT compute; CCE accumulate-DMAs tax the shared SDMA
  engines ~4x per byte; GpSimd tensor ops interfere ~2x with concurrent
  DVE ops - all three were measured and rejected.
- 12 segments ship as fp16 so their DVE subtracts run in 2x mode; the
  other 20 ship as fp8 e4m3 (1x subs) to keep HBM bytes down (5.5
  MiB/core). End-to-end quantization error ~4e-4 vs the fp32 reference.
- Squares: ACT does 28 segments (Square activation, 1 elem/cyc; Sin and
  Square share one trig_and_small table load), DVE squares the final
  fp8 group from its fp16 diffs in 2x mode to shorten the tail.
- Per-row weighted reduction on TensorE: 32 matmuls of
  psum[1,512] += w[:,n].T @ sq[:, n*512:(n+1)*512] with the per-row
  weight in a [128,1] fp16 stationary, then one DVE reduce of [1,512]
  and a 4-byte result DMA. Host sums the 8 per-core partials.
- Weights: ea/attr/invf (fp16) DMA after the first input piece; the
  half-angle identity 1-cos(x) = 2*sin^2(x/2) runs Sin on ACT with the
  factor 2 folded into the host-precomputed inv_freq tile.
"""

import math

import numpy as np

B, D = 32768, 512
M = 8  # cores
BS = B // M  # 4096 rows per core
P = 128  # SBUF partitions
NSEG = BS // P  # 32 row-segments of 512 per partition
NATTR = 6
GRP = 4  # segs per subtract/square group
F16SEG = 12  # segs 0..11 ship fp16; 12..31 ship fp8
# Tiny first piece: the first DVE subtract waits for the whole first
# piece pair, so a 2-seg pair (0.5 MiB) starts compute ~4us earlier
# than the 8-seg pair did (measured via the cumulative delivery curve).
PIECES16 = [2, 6, 4]  # fp16 DMA piece sizes (segs)
PIECES8 = [8, 8, 4]  # fp8 DMA piece sizes (segs)
DVE_SQ_GROUPS = (7,)  # square groups on DVE (rest on ACT)

_cache: dict = {}


def _build():
    import concourse.bacc as bacc
    import concourse.mybir as mybir
    import concourse.tile as tile

    nc = bacc.Bacc(
        "TRN2",
        debug=False,
        enable_asserts=False,
        num_devices=M,
    )
    f32 = mybir.dt.float32
    f16 = mybir.dt.float16
    f8 = mybir.dt.float8e4

    n16 = P * F16SEG  # rows shipped as fp16
    n8 = P * (NSEG - F16SEG)
    inp16 = nc.dram_tensor("inp16", [n16, D], f16, kind="ExternalInput").ap()
    lab16 = nc.dram_tensor("lab16", [n16, D], f16, kind="ExternalInput").ap()
    inp8 = nc.dram_tensor("inp8", [n8, D], f8, kind="ExternalInput").ap()
    lab8 = nc.dram_tensor("lab8", [n8, D], f8, kind="ExternalInput").ap()
    ea = nc.dram_tensor("ea", [BS, 3], f16, kind="ExternalInput").ap()
    attr = nc.dram_tensor("attr", [BS, NATTR], f16, kind="ExternalInput").ap()
    invf = nc.dram_tensor("invf", [P, NSEG * NATTR], f16, kind="ExternalInput").ap()
    out = nc.dram_tensor("out", [1, 1], f32, kind="ExternalOutput").ap()

    # host packs per-partition rows n in [0,12) into the fp16 tensors and
    # n in [12,32) into the fp8 tensors, each (p n) d contiguous.
    i16_v = inp16.rearrange("(p n) d -> p n d", p=P)  # [128, 12, 512]
    l16_v = lab16.rearrange("(p n) d -> p n d", p=P)
    i8_v = inp8.rearrange("(p n) d -> p n d", p=P)  # [128, 20, 512]
    l8_v = lab8.rearrange("(p n) d -> p n d", p=P)
    ea_v = ea.rearrange("(p n) t -> p n t", p=P)
    attr_v = attr.rearrange("(p n) c -> p n c", p=P)

    ADD = mybir.AluOpType.add
    MULT = mybir.AluOpType.mult
    AXX = mybir.AxisListType.X

    with tile.TileContext(nc) as tc:
        with (
            tc.tile_pool(name="big", bufs=1) as big,
            tc.tile_pool(name="small", bufs=1) as small,
            tc.tile_pool(name="psum", bufs=1, space="PSUM") as psum,
        ):
            in16_t = big.tile([P, F16SEG * D], f16)
            la16_t = big.tile([P, F16SEG * D], f16)
            in8_t = big.tile([P, (NSEG - F16SEG) * D], f8)
            la8_t = big.tile([P, (NSEG - F16SEG) * D], f8)
            diff16 = big.tile([P, NSEG * D], f16)
            sq_t = big.tile([P, NSEG * D], f16)
            acc = psum.tile([1, D], f32)

            def seg3(t, s0, n):
                return t[:, s0 * D : (s0 + n) * D].rearrange(
                    "p (n d) -> p n d", d=D
                )

            # ---- sync ring: fp16 pair 0, weights, rest of the pieces ----
            s = 0
            for k, npcs in enumerate(PIECES16):
                nc.sync.dma_start(seg3(in16_t, s, npcs), i16_v[:, s : s + npcs, :])
                nc.sync.dma_start(seg3(la16_t, s, npcs), l16_v[:, s : s + npcs, :])
                if k == 0:
                    ea_t = small.tile([P, NSEG * 3], f16)
                    nc.sync.dma_start(
                        ea_t[:].rearrange("p (n t) -> p n t", t=3), ea_v
                    )
                    attr_t = small.tile([P, NSEG * NATTR], f16)
                    nc.sync.dma_start(
                        attr_t[:].rearrange("p (n c) -> p n c", c=NATTR), attr_v
                    )
                    invf_t = small.tile([P, NSEG * NATTR], f16)
                    nc.sync.dma_start(invf_t[:], invf)
                s += npcs
            s = 0
            for npcs in PIECES8:
                nc.sync.dma_start(seg3(in8_t, s, npcs), i8_v[:, s : s + npcs, :])
                nc.sync.dma_start(seg3(la8_t, s, npcs), l8_v[:, s : s + npcs, :])
                s += npcs

            # ---- weights ----
            half = small.tile([P, NSEG * 3], f16)
            nc.vector.tensor_scalar(
                half[:], ea_t[:], 0.5, math.pi, MULT, mybir.AluOpType.min
            )
            nc.vector.tensor_scalar_max(half[:], half[:], -math.pi)
            sin_t = small.tile([P, NSEG * 3], f16)
            nc.scalar.activation(
                sin_t[:], half[:], mybir.ActivationFunctionType.Sin
            )
            sin2 = small.tile([P, NSEG * 3], f16)
            nc.vector.tensor_mul(sin2[:], sin_t[:], sin_t[:])
            csum = small.tile([P, NSEG], f32)
            nc.vector.tensor_reduce(
                csum[:], sin2[:].rearrange("p (n t) -> p n t", t=3), axis=AXX, op=ADD
            )
            awe = small.tile([P, NSEG * NATTR], f16)
            nc.vector.tensor_mul(awe[:], attr_t[:], invf_t[:])
            attrw = small.tile([P, NSEG], f32)
            nc.vector.tensor_reduce(
                attrw[:],
                awe[:].rearrange("p (n c) -> p n c", c=NATTR),
                axis=AXX,
                op=ADD,
            )
            w16 = small.tile([P, NSEG], f16)
            nc.vector.tensor_mul(w16[:], csum[:], attrw[:])

            mm = [0]

            def matmuls(g):
                for n_ in range(g * GRP, (g + 1) * GRP):
                    nc.tensor.matmul(
                        acc[:],
                        w16[:, n_ : n_ + 1],
                        sq_t[:, n_ * D : (n_ + 1) * D],
                        start=(mm[0] == 0),
                        stop=(mm[0] == NSEG - 1),
                    )
                    mm[0] += 1

            # ---- subtracts: fp16 per DMA piece, fp8 per 4-seg group ----
            s = 0
            for npcs in PIECES16:
                a, b = s * D, (s + npcs) * D
                nc.vector.tensor_add(
                    diff16[:, a:b], in16_t[:, a:b], la16_t[:, a:b]
                )
                s += npcs
            for g in range(F16SEG // GRP, NSEG // GRP):
                a = g * GRP * D
                b = (g + 1) * GRP * D
                a8 = a - F16SEG * D
                b8 = b - F16SEG * D
                nc.vector.tensor_add(
                    diff16[:, a:b], in8_t[:, a8:b8], la8_t[:, a8:b8]
                )

            # ---- square + matmul per 4-seg group ----
            for g in range(NSEG // GRP):
                a = g * GRP * D
                b = (g + 1) * GRP * D
                if g in DVE_SQ_GROUPS:
                    nc.vector.tensor_mul(
                        sq_t[:, a:b], diff16[:, a:b], diff16[:, a:b]
                    )
                else:
                    nc.scalar.activation(
                        sq_t[:, a:b],
                        diff16[:, a:b],
                        mybir.ActivationFunctionType.Square,
                    )
                matmuls(g)
            assert mm[0] == NSEG

            # ---- epilogue ----
            part = small.tile([1, 1], f32)
            nc.vector.tensor_reduce(part[:], acc[:], axis=AXX, op=ADD)
            nc.sync.dma_start(out, part[:])

    nc.compile()
    return nc


def get_nc():
    if "nc" not in _cache:
        _cache["nc"] = _build()
    return _cache["nc"]


def make_in_maps(inp, label, ea, attribute, attribute_num):
    import ml_dtypes

    f8 = ml_dtypes.float8_e4m3
    inv_freq2 = (
        2.0
        * np.asarray(attribute_num, dtype=np.float64).sum()
        / np.asarray(attribute_num, dtype=np.float64)
    ).astype(np.float16)
    invf_tiled = np.ascontiguousarray(
        np.broadcast_to(np.tile(inv_freq2, NSEG), (P, NSEG * NATTR))
    )
    inp32 = np.asarray(inp, dtype=np.float32)
    lab32 = -np.asarray(label, dtype=np.float32)
    ea16 = np.asarray(ea, dtype=np.float16)
    attr16 = np.asarray(attribute, dtype=np.float16)
    in_maps = []
    for c in range(M):
        s = slice(c * BS, (c + 1) * BS)
        iv = inp32[s].reshape(P, NSEG, D)
        lv = lab32[s].reshape(P, NSEG, D)
        in_maps.append(
            {
                "inp16": np.ascontiguousarray(
                    iv[:, :F16SEG].reshape(-1, D).astype(np.float16)
                ),
                "lab16": np.ascontiguousarray(
                    lv[:, :F16SEG].reshape(-1, D).astype(np.float16)
                ),
                "inp8": np.ascontiguousarray(
                    iv[:, F16SEG:].reshape(-1, D).astype(f8)
                ),
                "lab8": np.ascontiguousarray(
                    lv[:, F16SEG:].reshape(-1, D).astype(f8)
                ),
                "ea": np.ascontiguousarray(ea16[s]),
                "attr": np.ascontiguousarray(attr16[s]),
                "invf": invf_tiled,
            }
        )
    return in_maps


def kernel(inp, label, ea, attribute, attribute_num, batch_size=None, **_ignored):
    from concourse import bass_utils

    nc = get_nc()
    in_maps = make_in_maps(
        np.asarray(inp, dtype=np.float32),
        np.asarray(label, dtype=np.float32),
        np.asarray(ea, dtype=np.float32),
        np.asarray(attribute, dtype=np.int32),
        np.asarray(attribute_num, dtype=np.float32),
    )
    res = bass_utils.run_bass_kernel_spmd(nc, in_maps, core_ids=list(range(M)))
    total = 0.0
    for r in res.results:
        total += float(np.asarray(r["out"], dtype=np.float64)[0, 0])
    return np.float32(total / (B * D))


# revision 22
# speedup vs baseline: 1.0038x; 1.0038x over previous
"""Weighted-MSE loss (Euler-angle + attribute weights) on 8 trn2 NeuronCores.

loss = mean(weight * (inp - label)^2),
  weight[i] = (sum_j 1-cos(ea[i,j])) * (sum_c attribute[i,c] * inv_freq[c])

Pure data-parallel over the batch dim; each of the 8 cores gets 4096 rows
(32 segments of 512 columns per SBUF partition: partition p holds rows
p*32..p*32+31). label is negated on host so every subtract is an ADD.

Structure (settled by profiling several designs on hardware):
- All DMAs ride the sync ring (HWDGE issued by the otherwise-idle Sync
  engine, ~390 GB/s aggregate). DMAs issued via the scalar ring
  serialize with ACT compute; CCE accumulate-DMAs tax the shared SDMA
  engines ~4x per byte; GpSimd tensor ops interfere ~2x with concurrent
  DVE ops - all three were measured and rejected.
- 12 segments ship as fp16 so their DVE subtracts run in 2x mode; the
  other 20 ship as fp8 e4m3 (1x subs) to keep HBM bytes down (5.5
  MiB/core). End-to-end quantization error ~4e-4 vs the fp32 reference.
- Squares: ACT does 28 segments (Square activation, 1 elem/cyc; Sin and
  Square share one trig_and_small table load), DVE squares the final
  fp8 group from its fp16 diffs in 2x mode to shorten the tail.
- Per-row weighted reduction on TensorE: 32 matmuls of
  psum[1,512] += w[:,n].T @ sq[:, n*512:(n+1)*512] with the per-row
  weight in a [128,1] fp16 stationary, then one DVE reduce of [1,512]
  and a 4-byte result DMA. Host sums the 8 per-core partials.
- Weights: ea/attr/invf (fp16) DMA after the first input piece; the
  half-angle identity 1-cos(x) = 2*sin^2(x/2) runs Sin on ACT with the
  factor 2 folded into the host-precomputed inv_freq tile.
"""

import math

import numpy as np

B, D = 32768, 512
M = 8  # cores
BS = B // M  # 4096 rows per core
P = 128  # SBUF partitions
NSEG = BS // P  # 32 row-segments of 512 per partition
NATTR = 6
GRP = 4  # segs per subtract/square group
F16SEG = 12  # segs 0..11 ship fp16; 12..31 ship fp8
PIECES16 = [8, 4]  # fp16 DMA piece sizes (segs)
PIECES8 = [8, 8, 4]  # fp8 DMA piece sizes (segs)
DVE_SQ_GROUPS = (7,)  # square groups on DVE (rest on ACT)

_cache: dict = {}


def _build():
    import concourse.bacc as bacc
    import concourse.mybir as mybir
    import concourse.tile as tile

    nc = bacc.Bacc(
        "TRN2",
        debug=False,
        enable_asserts=False,
        num_devices=M,
    )
    f32 = mybir.dt.float32
    f16 = mybir.dt.float16
    f8 = mybir.dt.float8e4

    n16 = P * F16SEG  # rows shipped as fp16
    n8 = P * (NSEG - F16SEG)
    inp16 = nc.dram_tensor("inp16", [n16, D], f16, kind="ExternalInput").ap()
    lab16 = nc.dram_tensor("lab16", [n16, D], f16, kind="ExternalInput").ap()
    inp8 = nc.dram_tensor("inp8", [n8, D], f8, kind="ExternalInput").ap()
    lab8 = nc.dram_tensor("lab8", [n8, D], f8, kind="ExternalInput").ap()
    ea = nc.dram_tensor("ea", [BS, 3], f16, kind="ExternalInput").ap()
    attr = nc.dram_tensor("attr", [BS, NATTR], f16, kind="ExternalInput").ap()
    invf = nc.dram_tensor("invf", [P, NSEG * NATTR], f16, kind="ExternalInput").ap()
    out = nc.dram_tensor("out", [1, 1], f32, kind="ExternalOutput").ap()

    # host packs per-partition rows n in [0,12) into the fp16 tensors and
    # n in [12,32) into the fp8 tensors, each (p n) d contiguous.
    i16_v = inp16.rearrange("(p n) d -> p n d", p=P)  # [128, 12, 512]
    l16_v = lab16.rearrange("(p n) d -> p n d", p=P)
    i8_v = inp8.rearrange("(p n) d -> p n d", p=P)  # [128, 20, 512]
    l8_v = lab8.rearrange("(p n) d -> p n d", p=P)
    ea_v = ea.rearrange("(p n) t -> p n t", p=P)
    attr_v = attr.rearrange("(p n) c -> p n c", p=P)

    ADD = mybir.AluOpType.add
    MULT = mybir.AluOpType.mult
    AXX = mybir.AxisListType.X

    with tile.TileContext(nc) as tc:
        with (
            tc.tile_pool(name="big", bufs=1) as big,
            tc.tile_pool(name="small", bufs=1) as small,
            tc.tile_pool(name="psum", bufs=1, space="PSUM") as psum,
        ):
            in16_t = big.tile([P, F16SEG * D], f16)
            la16_t = big.tile([P, F16SEG * D], f16)
            in8_t = big.tile([P, (NSEG - F16SEG) * D], f8)
            la8_t = big.tile([P, (NSEG - F16SEG) * D], f8)
            diff16 = big.tile([P, NSEG * D], f16)
            sq_t = big.tile([P, NSEG * D], f16)
            acc = psum.tile([1, D], f32)

            def seg3(t, s0, n):
                return t[:, s0 * D : (s0 + n) * D].rearrange(
                    "p (n d) -> p n d", d=D
                )

            # ---- sync ring: fp16 pair 0, weights, rest of the pieces ----
            s = 0
            for k, npcs in enumerate(PIECES16):
                nc.sync.dma_start(seg3(in16_t, s, npcs), i16_v[:, s : s + npcs, :])
                nc.sync.dma_start(seg3(la16_t, s, npcs), l16_v[:, s : s + npcs, :])
                if k == 0:
                    ea_t = small.tile([P, NSEG * 3], f16)
                    nc.sync.dma_start(
                        ea_t[:].rearrange("p (n t) -> p n t", t=3), ea_v
                    )
                    attr_t = small.tile([P, NSEG * NATTR], f16)
                    nc.sync.dma_start(
                        attr_t[:].rearrange("p (n c) -> p n c", c=NATTR), attr_v
                    )
                    invf_t = small.tile([P, NSEG * NATTR], f16)
                    nc.sync.dma_start(invf_t[:], invf)
                s += npcs
            s = 0
            for npcs in PIECES8:
                nc.sync.dma_start(seg3(in8_t, s, npcs), i8_v[:, s : s + npcs, :])
                nc.sync.dma_start(seg3(la8_t, s, npcs), l8_v[:, s : s + npcs, :])
                s += npcs

            # ---- weights ----
            half = small.tile([P, NSEG * 3], f16)
            nc.vector.tensor_scalar(
                half[:], ea_t[:], 0.5, math.pi, MULT, mybir.AluOpType.min
            )
            nc.vector.tensor_scalar_max(half[:], half[:], -math.pi)
            sin_t = small.tile([P, NSEG * 3], f16)
            nc.scalar.activation(
                sin_t[:], half[:], mybir.ActivationFunctionType.Sin
            )
            sin2 = small.tile([P, NSEG * 3], f16)
            nc.vector.tensor_mul(sin2[:], sin_t[:], sin_t[:])
            csum = small.tile([P, NSEG], f32)
            nc.vector.tensor_reduce(
                csum[:], sin2[:].rearrange("p (n t) -> p n t", t=3), axis=AXX, op=ADD
            )
            awe = small.tile([P, NSEG * NATTR], f16)
            nc.vector.tensor_mul(awe[:], attr_t[:], invf_t[:])
            attrw = small.tile([P, NSEG], f32)
            nc.vector.tensor_reduce(
                attrw[:],
                awe[:].rearrange("p (n c) -> p n c", c=NATTR),
                axis=AXX,
                op=ADD,
            )
            w16 = small.tile([P, NSEG], f16)
            nc.vector.tensor_mul(w16[:], csum[:], attrw[:])

            mm = [0]

            def matmuls(g):
                for n_ in range(g * GRP, (g + 1) * GRP):
                    nc.tensor.matmul(
                        acc[:],
                        w16[:, n_ : n_ + 1],
                        sq_t[:, n_ * D : (n_ + 1) * D],
                        start=(mm[0] == 0),
                        stop=(mm[0] == NSEG - 1),
                    )
                    mm[0] += 1

            # ---- subtract + square + matmul per 4-seg group ----
            for g in range(NSEG // GRP):
                a = g * GRP * D
                b = (g + 1) * GRP * D
                if g * GRP < F16SEG:  # fp16 region: DVE sub at 2x
                    nc.vector.tensor_add(
                        diff16[:, a:b], in16_t[:, a:b], la16_t[:, a:b]
                    )
                else:  # fp8 region: DVE sub at 1x
                    a8 = a - F16SEG * D
                    b8 = b - F16SEG * D
                    nc.vector.tensor_add(
                        diff16[:, a:b], in8_t[:, a8:b8], la8_t[:, a8:b8]
                    )
                if g in DVE_SQ_GROUPS:
                    nc.vector.tensor_mul(
                        sq_t[:, a:b], diff16[:, a:b], diff16[:, a:b]
                    )
                else:
                    nc.scalar.activation(
                        sq_t[:, a:b],
                        diff16[:, a:b],
                        mybir.ActivationFunctionType.Square,
                    )
                matmuls(g)
            assert mm[0] == NSEG

            # ---- epilogue ----
            part = small.tile([1, 1], f32)
            nc.vector.tensor_reduce(part[:], acc[:], axis=AXX, op=ADD)
            nc.sync.dma_start(out, part[:])

    nc.compile()
    return nc


def get_nc():
    if "nc" not in _cache:
        _cache["nc"] = _build()
    return _cache["nc"]


def make_in_maps(inp, label, ea, attribute, attribute_num):
    import ml_dtypes

    f8 = ml_dtypes.float8_e4m3
    inv_freq2 = (
        2.0
        * np.asarray(attribute_num, dtype=np.float64).sum()
        / np.asarray(attribute_num, dtype=np.float64)
    ).astype(np.float16)
    invf_tiled = np.ascontiguousarray(
        np.broadcast_to(np.tile(inv_freq2, NSEG), (P, NSEG * NATTR))
    )
    inp32 = np.asarray(inp, dtype=np.float32)
    lab32 = -np.asarray(label, dtype=np.float32)
    ea16 = np.asarray(ea, dtype=np.float16)
    attr16 = np.asarray(attribute, dtype=np.float16)
    in_maps = []
    for c in range(M):
        s = slice(c * BS, (c + 1) * BS)
        iv = inp32[s].reshape(P, NSEG, D)
        lv = lab32[s].reshape(P, NSEG, D)
        in_maps.append(
            {
                "inp16": np.ascontiguousarray(
                    iv[:, :F16SEG].reshape(-1, D).astype(np.float16)
                ),
                "lab16": np.ascontiguousarray(
                    lv[:, :F16SEG].reshape(-1, D).astype(np.float16)
                ),
                "inp8": np.ascontiguousarray(
                    iv[:, F16SEG:].reshape(-1, D).astype(f8)
                ),
                "lab8": np.ascontiguousarray(
                    lv[:, F16SEG:].reshape(-1, D).astype(f8)
                ),
                "ea": np.ascontiguousarray(ea16[s]),
                "attr": np.ascontiguousarray(attr16[s]),
                "invf": invf_tiled,
            }
        )
    return in_maps


def kernel(inp, label, ea, attribute, attribute_num, batch_size=None, **_ignored):
    from concourse import bass_utils

    nc = get_nc()
    in_maps = make_in_maps(
        np.asarray(inp, dtype=np.float32),
        np.asarray(label, dtype=np.float32),
        np.asarray(ea, dtype=np.float32),
        np.asarray(attribute, dtype=np.int32),
        np.asarray(attribute_num, dtype=np.float32),
    )
    res = bass_utils.run_bass_kernel_spmd(nc, in_maps, core_ids=list(range(M)))
    total = 0.0
    for r in res.results:
        total += float(np.asarray(r["out"], dtype=np.float64)[0, 0])
    return np.float32(total / (B * D))


# revision 24
# speedup vs baseline: 1.0895x; 1.0854x over previous
"""Weighted-MSE loss (Euler-angle + attribute weights) on 8 trn2 NeuronCores.

loss = mean(weight * (inp - label)^2),
  weight[i] = (sum_j 1-cos(ea[i,j])) * (sum_c attribute[i,c] * inv_freq[c])

Pure data-parallel over the batch dim; each of the 8 cores gets 4096 rows
(32 segments of 512 columns per SBUF partition: partition p holds rows
p*32..p*32+31). label is negated on host so every subtract is an ADD.

Structure (settled by profiling several designs on hardware):
- All DMAs ride the sync ring (HWDGE issued by the otherwise-idle Sync
  engine, ~390 GB/s aggregate). DMAs issued via the scalar ring
  serialize with ACT compute; CCE accumulate-DMAs tax the shared SDMA
  engines ~4x per byte; GpSimd tensor ops interfere ~2x with concurrent
  DVE ops - all three were measured and rejected.
- 12 segments ship as fp16 so their DVE subtracts run in 2x mode; the
  other 20 ship as fp8 e4m3 (1x subs) to keep HBM bytes down (5.5
  MiB/core). End-to-end quantization error ~4e-4 vs the fp32 reference.
- Squares: ACT does 28 segments (Square activation, 1 elem/cyc; Sin and
  Square share one trig_and_small table load), DVE squares the final
  fp8 group from its fp16 diffs in 2x mode to shorten the tail.
- Per-row weighted reduction on TensorE: 32 matmuls of
  psum[1,512] += w[:,n].T @ sq[:, n*512:(n+1)*512] with the per-row
  weight in a [128,1] fp16 stationary, then one DVE reduce of [1,512]
  and a 4-byte result DMA. Host sums the 8 per-core partials.
- Weights: ea/attr/invf (fp16) DMA after the first input piece; the
  half-angle identity 1-cos(x) = 2*sin^2(x/2) runs Sin on ACT with the
  factor 2 folded into the host-precomputed inv_freq tile.
"""

import math

import numpy as np

B, D = 32768, 512
M = 8  # cores
BS = B // M  # 4096 rows per core
P = 128  # SBUF partitions
NSEG = BS // P  # 32 row-segments of 512 per partition
NATTR = 6
GRP = 4  # segs per subtract/square group
F16SEG = 12  # segs 0..11 ship fp16; 12..31 ship fp8
PIECES16 = [8, 4]  # fp16 DMA piece sizes (segs)
PIECES8 = [8, 8, 4]  # fp8 DMA piece sizes (segs)
DVE_SQ_GROUPS = (7,)  # square groups on DVE (rest on ACT)

_cache: dict = {}


def _build():
    import concourse.bacc as bacc
    import concourse.mybir as mybir
    import concourse.tile as tile

    nc = bacc.Bacc(
        "TRN2",
        debug=False,
        enable_asserts=False,
        num_devices=M,
    )
    f32 = mybir.dt.float32
    f16 = mybir.dt.float16
    f8 = mybir.dt.float8e4

    n16 = P * F16SEG  # rows shipped as fp16
    n8 = P * (NSEG - F16SEG)
    inp16 = nc.dram_tensor("inp16", [n16, D], f16, kind="ExternalInput").ap()
    lab16 = nc.dram_tensor("lab16", [n16, D], f16, kind="ExternalInput").ap()
    inp8 = nc.dram_tensor("inp8", [n8, D], f8, kind="ExternalInput").ap()
    lab8 = nc.dram_tensor("lab8", [n8, D], f8, kind="ExternalInput").ap()
    ea = nc.dram_tensor("ea", [BS, 3], f16, kind="ExternalInput").ap()
    attr = nc.dram_tensor("attr", [BS, NATTR], f16, kind="ExternalInput").ap()
    invf = nc.dram_tensor("invf", [P, NSEG * NATTR], f16, kind="ExternalInput").ap()
    out = nc.dram_tensor("out", [1, 1], f32, kind="ExternalOutput").ap()

    # host packs per-partition rows n in [0,12) into the fp16 tensors and
    # n in [12,32) into the fp8 tensors, each (p n) d contiguous.
    i16_v = inp16.rearrange("(p n) d -> p n d", p=P)  # [128, 12, 512]
    l16_v = lab16.rearrange("(p n) d -> p n d", p=P)
    i8_v = inp8.rearrange("(p n) d -> p n d", p=P)  # [128, 20, 512]
    l8_v = lab8.rearrange("(p n) d -> p n d", p=P)
    ea_v = ea.rearrange("(p n) t -> p n t", p=P)
    attr_v = attr.rearrange("(p n) c -> p n c", p=P)

    ADD = mybir.AluOpType.add
    MULT = mybir.AluOpType.mult
    AXX = mybir.AxisListType.X

    with tile.TileContext(nc) as tc:
        with (
            tc.tile_pool(name="big", bufs=1) as big,
            tc.tile_pool(name="small", bufs=1) as small,
            tc.tile_pool(name="psum", bufs=1, space="PSUM") as psum,
        ):
            in16_t = big.tile([P, F16SEG * D], f16)
            la16_t = big.tile([P, F16SEG * D], f16)
            in8_t = big.tile([P, (NSEG - F16SEG) * D], f8)
            la8_t = big.tile([P, (NSEG - F16SEG) * D], f8)
            diff16 = big.tile([P, NSEG * D], f16)
            sq_t = big.tile([P, NSEG * D], f16)
            acc = psum.tile([1, D], f32)

            def seg3(t, s0, n):
                return t[:, s0 * D : (s0 + n) * D].rearrange(
                    "p (n d) -> p n d", d=D
                )

            # ---- sync ring: a small fp8 pair first (0.5 MiB lands ~4us
            # before a 2 MiB fp16 pair would, so DVE starts subtracting
            # earlier), then fp16 pieces + weights, then remaining fp8 ----
            nc.sync.dma_start(
                seg3(in8_t, 0, PIECES8[0]), i8_v[:, 0 : PIECES8[0], :]
            )
            nc.sync.dma_start(
                seg3(la8_t, 0, PIECES8[0]), l8_v[:, 0 : PIECES8[0], :]
            )
            s = 0
            for k, npcs in enumerate(PIECES16):
                nc.sync.dma_start(seg3(in16_t, s, npcs), i16_v[:, s : s + npcs, :])
                nc.sync.dma_start(seg3(la16_t, s, npcs), l16_v[:, s : s + npcs, :])
                if k == 0:
                    ea_t = small.tile([P, NSEG * 3], f16)
                    nc.sync.dma_start(
                        ea_t[:].rearrange("p (n t) -> p n t", t=3), ea_v
                    )
                    attr_t = small.tile([P, NSEG * NATTR], f16)
                    nc.sync.dma_start(
                        attr_t[:].rearrange("p (n c) -> p n c", c=NATTR), attr_v
                    )
                    invf_t = small.tile([P, NSEG * NATTR], f16)
                    nc.sync.dma_start(invf_t[:], invf)
                s += npcs
            s = PIECES8[0]
            for npcs in PIECES8[1:]:
                nc.sync.dma_start(seg3(in8_t, s, npcs), i8_v[:, s : s + npcs, :])
                nc.sync.dma_start(seg3(la8_t, s, npcs), l8_v[:, s : s + npcs, :])
                s += npcs

            # ---- weights ----
            half = small.tile([P, NSEG * 3], f16)
            nc.vector.tensor_scalar(
                half[:], ea_t[:], 0.5, math.pi, MULT, mybir.AluOpType.min
            )
            nc.vector.tensor_scalar_max(half[:], half[:], -math.pi)
            sin_t = small.tile([P, NSEG * 3], f16)
            nc.scalar.activation(
                sin_t[:], half[:], mybir.ActivationFunctionType.Sin
            )
            sin2 = small.tile([P, NSEG * 3], f16)
            nc.vector.tensor_mul(sin2[:], sin_t[:], sin_t[:])
            csum = small.tile([P, NSEG], f32)
            nc.vector.tensor_reduce(
                csum[:], sin2[:].rearrange("p (n t) -> p n t", t=3), axis=AXX, op=ADD
            )
            awe = small.tile([P, NSEG * NATTR], f16)
            nc.vector.tensor_mul(awe[:], attr_t[:], invf_t[:])
            attrw = small.tile([P, NSEG], f32)
            nc.vector.tensor_reduce(
                attrw[:],
                awe[:].rearrange("p (n c) -> p n c", c=NATTR),
                axis=AXX,
                op=ADD,
            )
            w16 = small.tile([P, NSEG], f16)
            nc.vector.tensor_mul(w16[:], csum[:], attrw[:])

            mm = [0]

            def matmuls(g):
                for n_ in range(g * GRP, (g + 1) * GRP):
                    nc.tensor.matmul(
                        acc[:],
                        w16[:, n_ : n_ + 1],
                        sq_t[:, n_ * D : (n_ + 1) * D],
                        start=(mm[0] == 0),
                        stop=(mm[0] == NSEG - 1),
                    )
                    mm[0] += 1

            # ---- subtract + square + matmul per 4-seg group ----
            # group order follows data-delivery order: the first fp8
            # piece (groups 3,4), then fp16 (0-2), then fp8 (5-7)
            for g in (3, 4, 0, 1, 2, 5, 6, 7):
                a = g * GRP * D
                b = (g + 1) * GRP * D
                if g * GRP < F16SEG:  # fp16 region: DVE sub at 2x
                    nc.vector.tensor_add(
                        diff16[:, a:b], in16_t[:, a:b], la16_t[:, a:b]
                    )
                else:  # fp8 region: DVE sub at 1x
                    a8 = a - F16SEG * D
                    b8 = b - F16SEG * D
                    nc.vector.tensor_add(
                        diff16[:, a:b], in8_t[:, a8:b8], la8_t[:, a8:b8]
                    )
                if g in DVE_SQ_GROUPS:
                    nc.vector.tensor_mul(
                        sq_t[:, a:b], diff16[:, a:b], diff16[:, a:b]
                    )
                else:
                    nc.scalar.activation(
                        sq_t[:, a:b],
                        diff16[:, a:b],
                        mybir.ActivationFunctionType.Square,
                    )
                matmuls(g)
            assert mm[0] == NSEG

            # ---- epilogue ----
            part = small.tile([1, 1], f32)
            nc.vector.tensor_reduce(part[:], acc[:], axis=AXX, op=ADD)
            nc.sync.dma_start(out, part[:])

    nc.compile()
    return nc


def get_nc():
    if "nc" not in _cache:
        _cache["nc"] = _build()
    return _cache["nc"]


def make_in_maps(inp, label, ea, attribute, attribute_num):
    import ml_dtypes

    f8 = ml_dtypes.float8_e4m3
    inv_freq2 = (
        2.0
        * np.asarray(attribute_num, dtype=np.float64).sum()
        / np.asarray(attribute_num, dtype=np.float64)
    ).astype(np.float16)
    invf_tiled = np.ascontiguousarray(
        np.broadcast_to(np.tile(inv_freq2, NSEG), (P, NSEG * NATTR))
    )
    inp32 = np.asarray(inp, dtype=np.float32)
    lab32 = -np.asarray(label, dtype=np.float32)
    ea16 = np.asarray(ea, dtype=np.float16)
    attr16 = np.asarray(attribute, dtype=np.float16)
    in_maps = []
    for c in range(M):
        s = slice(c * BS, (c + 1) * BS)
        iv = inp32[s].reshape(P, NSEG, D)
        lv = lab32[s].reshape(P, NSEG, D)
        in_maps.append(
            {
                "inp16": np.ascontiguousarray(
                    iv[:, :F16SEG].reshape(-1, D).astype(np.float16)
                ),
                "lab16": np.ascontiguousarray(
                    lv[:, :F16SEG].reshape(-1, D).astype(np.float16)
                ),
                "inp8": np.ascontiguousarray(
                    iv[:, F16SEG:].reshape(-1, D).astype(f8)
                ),
                "lab8": np.ascontiguousarray(
                    lv[:, F16SEG:].reshape(-1, D).astype(f8)
                ),
                "ea": np.ascontiguousarray(ea16[s]),
                "attr": np.ascontiguousarray(attr16[s]),
                "invf": invf_tiled,
            }
        )
    return in_maps


def kernel(inp, label, ea, attribute, attribute_num, batch_size=None, **_ignored):
    from concourse import bass_utils

    nc = get_nc()
    in_maps = make_in_maps(
        np.asarray(inp, dtype=np.float32),
        np.asarray(label, dtype=np.float32),
        np.asarray(ea, dtype=np.float32),
        np.asarray(attribute, dtype=np.int32),
        np.asarray(attribute_num, dtype=np.float32),
    )
    res = bass_utils.run_bass_kernel_spmd(nc, in_maps, core_ids=list(range(M)))
    total = 0.0
    for r in res.results:
        total += float(np.asarray(r["out"], dtype=np.float64)[0, 0])
    return np.float32(total / (B * D))


# revision 25
# speedup vs baseline: 1.0898x; 1.0003x over previous
"""Weighted-MSE loss (Euler-angle + attribute weights) on 8 trn2 NeuronCores.

loss = mean(weight * (inp - label)^2),
  weight[i] = (sum_j 1-cos(ea[i,j])) * (sum_c attribute[i,c] * inv_freq[c])

Pure data-parallel over the batch dim; each of the 8 cores gets 4096 rows
(32 segments of 512 columns per SBUF partition: partition p holds rows
p*32..p*32+31). label is negated on host so every subtract is an ADD.

Structure (settled by profiling several designs on hardware):
- All DMAs ride the sync ring (HWDGE issued by the otherwise-idle Sync
  engine, ~390 GB/s aggregate). DMAs issued via the scalar ring
  serialize with ACT compute; CCE accumulate-DMAs tax the shared SDMA
  engines ~4x per byte; GpSimd tensor ops interfere ~2x with concurrent
  DVE ops - all three were measured and rejected.
- 12 segments ship as fp16 so their DVE subtracts run in 2x mode; the
  other 20 ship as fp8 e4m3 (1x subs) to keep HBM bytes down (5.5
  MiB/core). End-to-end quantization error ~4e-4 vs the fp32 reference.
- Squares: ACT does 28 segments (Square activation, 1 elem/cyc; Sin and
  Square share one trig_and_small table load), DVE squares the final
  fp8 group from its fp16 diffs in 2x mode to shorten the tail.
- Per-row weighted reduction on TensorE: 32 matmuls of
  psum[1,512] += w[:,n].T @ sq[:, n*512:(n+1)*512] with the per-row
  weight in a [128,1] fp16 stationary, then one DVE reduce of [1,512]
  and a 4-byte result DMA. Host sums the 8 per-core partials.
- Weights: ea/attr/invf (fp16) DMA after the first input piece; the
  half-angle identity 1-cos(x) = 2*sin^2(x/2) runs Sin on ACT with the
  factor 2 folded into the host-precomputed inv_freq tile.
"""

import math

import numpy as np

B, D = 32768, 512
M = 8  # cores
BS = B // M  # 4096 rows per core
P = 128  # SBUF partitions
NSEG = BS // P  # 32 row-segments of 512 per partition
NATTR = 6
GRP = 4  # segs per subtract/square group
F16SEG = 12  # segs 0..11 ship fp16; 12..31 ship fp8
PIECES16 = [4, 4, 4]  # fp16 DMA piece sizes (segs; 1 MiB pairs land
# every ~2.5us, matching the 4-seg subtract groups so DVE doesn't stall
# on a single 2 MiB pair mid-stream)
PIECES8 = [8, 8, 4]  # fp8 DMA piece sizes (segs)
DVE_SQ_GROUPS = (7,)  # square groups on DVE (rest on ACT)

_cache: dict = {}


def _build():
    import concourse.bacc as bacc
    import concourse.mybir as mybir
    import concourse.tile as tile

    nc = bacc.Bacc(
        "TRN2",
        debug=False,
        enable_asserts=False,
        num_devices=M,
    )
    f32 = mybir.dt.float32
    f16 = mybir.dt.float16
    f8 = mybir.dt.float8e4

    n16 = P * F16SEG  # rows shipped as fp16
    n8 = P * (NSEG - F16SEG)
    inp16 = nc.dram_tensor("inp16", [n16, D], f16, kind="ExternalInput").ap()
    lab16 = nc.dram_tensor("lab16", [n16, D], f16, kind="ExternalInput").ap()
    inp8 = nc.dram_tensor("inp8", [n8, D], f8, kind="ExternalInput").ap()
    lab8 = nc.dram_tensor("lab8", [n8, D], f8, kind="ExternalInput").ap()
    ea = nc.dram_tensor("ea", [BS, 3], f16, kind="ExternalInput").ap()
    attr = nc.dram_tensor("attr", [BS, NATTR], f16, kind="ExternalInput").ap()
    invf = nc.dram_tensor("invf", [P, NSEG * NATTR], f16, kind="ExternalInput").ap()
    out = nc.dram_tensor("out", [1, 1], f32, kind="ExternalOutput").ap()

    # host packs per-partition rows n in [0,12) into the fp16 tensors and
    # n in [12,32) into the fp8 tensors, each (p n) d contiguous.
    i16_v = inp16.rearrange("(p n) d -> p n d", p=P)  # [128, 12, 512]
    l16_v = lab16.rearrange("(p n) d -> p n d", p=P)
    i8_v = inp8.rearrange("(p n) d -> p n d", p=P)  # [128, 20, 512]
    l8_v = lab8.rearrange("(p n) d -> p n d", p=P)
    ea_v = ea.rearrange("(p n) t -> p n t", p=P)
    attr_v = attr.rearrange("(p n) c -> p n c", p=P)

    ADD = mybir.AluOpType.add
    MULT = mybir.AluOpType.mult
    AXX = mybir.AxisListType.X

    with tile.TileContext(nc) as tc:
        with (
            tc.tile_pool(name="big", bufs=1) as big,
            tc.tile_pool(name="small", bufs=1) as small,
            tc.tile_pool(name="psum", bufs=1, space="PSUM") as psum,
        ):
            in16_t = big.tile([P, F16SEG * D], f16)
            la16_t = big.tile([P, F16SEG * D], f16)
            in8_t = big.tile([P, (NSEG - F16SEG) * D], f8)
            la8_t = big.tile([P, (NSEG - F16SEG) * D], f8)
            diff16 = big.tile([P, NSEG * D], f16)
            sq_t = big.tile([P, NSEG * D], f16)
            acc = psum.tile([1, D], f32)

            def seg3(t, s0, n):
                return t[:, s0 * D : (s0 + n) * D].rearrange(
                    "p (n d) -> p n d", d=D
                )

            # ---- sync ring: a small fp8 pair first (0.5 MiB lands ~4us
            # before a 2 MiB fp16 pair would, so DVE starts subtracting
            # earlier), then fp16 pieces + weights, then remaining fp8 ----
            nc.sync.dma_start(
                seg3(in8_t, 0, PIECES8[0]), i8_v[:, 0 : PIECES8[0], :]
            )
            nc.sync.dma_start(
                seg3(la8_t, 0, PIECES8[0]), l8_v[:, 0 : PIECES8[0], :]
            )
            s = 0
            for k, npcs in enumerate(PIECES16):
                nc.sync.dma_start(seg3(in16_t, s, npcs), i16_v[:, s : s + npcs, :])
                nc.sync.dma_start(seg3(la16_t, s, npcs), l16_v[:, s : s + npcs, :])
                if k == 0:
                    ea_t = small.tile([P, NSEG * 3], f16)
                    nc.sync.dma_start(
                        ea_t[:].rearrange("p (n t) -> p n t", t=3), ea_v
                    )
                    attr_t = small.tile([P, NSEG * NATTR], f16)
                    nc.sync.dma_start(
                        attr_t[:].rearrange("p (n c) -> p n c", c=NATTR), attr_v
                    )
                    invf_t = small.tile([P, NSEG * NATTR], f16)
                    nc.sync.dma_start(invf_t[:], invf)
                s += npcs
            s = PIECES8[0]
            for npcs in PIECES8[1:]:
                nc.sync.dma_start(seg3(in8_t, s, npcs), i8_v[:, s : s + npcs, :])
                nc.sync.dma_start(seg3(la8_t, s, npcs), l8_v[:, s : s + npcs, :])
                s += npcs

            # ---- weights ----
            half = small.tile([P, NSEG * 3], f16)
            nc.vector.tensor_scalar(
                half[:], ea_t[:], 0.5, math.pi, MULT, mybir.AluOpType.min
            )
            nc.vector.tensor_scalar_max(half[:], half[:], -math.pi)
            sin_t = small.tile([P, NSEG * 3], f16)
            nc.scalar.activation(
                sin_t[:], half[:], mybir.ActivationFunctionType.Sin
            )
            sin2 = small.tile([P, NSEG * 3], f16)
            nc.vector.tensor_mul(sin2[:], sin_t[:], sin_t[:])
            csum = small.tile([P, NSEG], f32)
            nc.vector.tensor_reduce(
                csum[:], sin2[:].rearrange("p (n t) -> p n t", t=3), axis=AXX, op=ADD
            )
            awe = small.tile([P, NSEG * NATTR], f16)
            nc.vector.tensor_mul(awe[:], attr_t[:], invf_t[:])
            attrw = small.tile([P, NSEG], f32)
            nc.vector.tensor_reduce(
                attrw[:],
                awe[:].rearrange("p (n c) -> p n c", c=NATTR),
                axis=AXX,
                op=ADD,
            )
            w16 = small.tile([P, NSEG], f16)
            nc.vector.tensor_mul(w16[:], csum[:], attrw[:])

            mm = [0]

            def matmuls(g):
                for n_ in range(g * GRP, (g + 1) * GRP):
                    nc.tensor.matmul(
                        acc[:],
                        w16[:, n_ : n_ + 1],
                        sq_t[:, n_ * D : (n_ + 1) * D],
                        start=(mm[0] == 0),
                        stop=(mm[0] == NSEG - 1),
                    )
                    mm[0] += 1

            # ---- subtract + square + matmul per 4-seg group ----
            # group order follows data-delivery order: the first fp8
            # piece (groups 3,4), then fp16 (0-2), then fp8 (5-7)
            for g in (3, 4, 0, 1, 2, 5, 6, 7):
                a = g * GRP * D
                b = (g + 1) * GRP * D
                if g * GRP < F16SEG:  # fp16 region: DVE sub at 2x
                    nc.vector.tensor_add(
                        diff16[:, a:b], in16_t[:, a:b], la16_t[:, a:b]
                    )
                else:  # fp8 region: DVE sub at 1x
                    a8 = a - F16SEG * D
                    b8 = b - F16SEG * D
                    nc.vector.tensor_add(
                        diff16[:, a:b], in8_t[:, a8:b8], la8_t[:, a8:b8]
                    )
                if g in DVE_SQ_GROUPS:
                    nc.vector.tensor_mul(
                        sq_t[:, a:b], diff16[:, a:b], diff16[:, a:b]
                    )
                else:
                    nc.scalar.activation(
                        sq_t[:, a:b],
                        diff16[:, a:b],
                        mybir.ActivationFunctionType.Square,
                    )
                matmuls(g)
            assert mm[0] == NSEG

            # ---- epilogue ----
            part = small.tile([1, 1], f32)
            nc.vector.tensor_reduce(part[:], acc[:], axis=AXX, op=ADD)
            nc.sync.dma_start(out, part[:])

    nc.compile()
    return nc


def get_nc():
    if "nc" not in _cache:
        _cache["nc"] = _build()
    return _cache["nc"]


def make_in_maps(inp, label, ea, attribute, attribute_num):
    import ml_dtypes

    f8 = ml_dtypes.float8_e4m3
    inv_freq2 = (
        2.0
        * np.asarray(attribute_num, dtype=np.float64).sum()
        / np.asarray(attribute_num, dtype=np.float64)
    ).astype(np.float16)
    invf_tiled = np.ascontiguousarray(
        np.broadcast_to(np.tile(inv_freq2, NSEG), (P, NSEG * NATTR))
    )
    inp32 = np.asarray(inp, dtype=np.float32)
    lab32 = -np.asarray(label, dtype=np.float32)
    ea16 = np.asarray(ea, dtype=np.float16)
    attr16 = np.asarray(attribute, dtype=np.float16)
    in_maps = []
    for c in range(M):
        s = slice(c * BS, (c + 1) * BS)
        iv = inp32[s].reshape(P, NSEG, D)
        lv = lab32[s].reshape(P, NSEG, D)
        in_maps.append(
            {
                "inp16": np.ascontiguousarray(
                    iv[:, :F16SEG].reshape(-1, D).astype(np.float16)
                ),
                "lab16": np.ascontiguousarray(
                    lv[:, :F16SEG].reshape(-1, D).astype(np.float16)
                ),
                "inp8": np.ascontiguousarray(
                    iv[:, F16SEG:].reshape(-1, D).astype(f8)
                ),
                "lab8": np.ascontiguousarray(
                    lv[:, F16SEG:].reshape(-1, D).astype(f8)
                ),
                "ea": np.ascontiguousarray(ea16[s]),
                "attr": np.ascontiguousarray(attr16[s]),
                "invf": invf_tiled,
            }
        )
    return in_maps


def kernel(inp, label, ea, attribute, attribute_num, batch_size=None, **_ignored):
    from concourse import bass_utils

    nc = get_nc()
    in_maps = make_in_maps(
        np.asarray(inp, dtype=np.float32),
        np.asarray(label, dtype=np.float32),
        np.asarray(ea, dtype=np.float32),
        np.asarray(attribute, dtype=np.int32),
        np.asarray(attribute_num, dtype=np.float32),
    )
    res = bass_utils.run_bass_kernel_spmd(nc, in_maps, core_ids=list(range(M)))
    total = 0.0
    for r in res.results:
        total += float(np.asarray(r["out"], dtype=np.float64)[0, 0])
    return np.float32(total / (B * D))
